# revision 5
# baseline (speedup 1.0000x reference)
"""Per-pixel kernel-lookup conv for trn2, data-parallel over batch on 8 cores.

Per core (one image): conv against all 128 kernels via 2 matmuls per
512-col chunk (A: K=96 = 16ch x {(dy,0),(dy,1)} materialized shifts;
B: K=48 dx=2 via +2 column offset on the dx=0 rows), ACT evacuates
PSUM->fp16, DVE does (idx==j)*conv select, a sliding-window ones-column
matmul partition-reduces each chunk into its own PSUM row, single ACT
evac + DMA out at the end.
"""
import os

import numpy as np

DT16 = os.environ.get("V2DT", "fp16")  # fp16 | bf16 for matmul operands
RAST = 126 * 128          # valid output raster (126 rows x 128 padded cols)
CH = 512                  # chunk cols (one PSUM bank of f32)
NCH = 32                  # chunks (covers 16384 >= RAST, tail is don't-care)
TOT = CH * NCH            # 16384
BPAD = 16512              # btileA cols: TOT + 2 (dx shift) rounded up
_NC_CACHE = {}

_OFFS = [(0, 0), (1, 0), (2, 0), (0, 1), (1, 1), (2, 1)]  # btileA row groups


def _split_waits_json(bj: bytes) -> bytes:
    """Walrus rejects >4 sync-waits per instruction (and ~2 on Matmult).
    Split excess waits onto same-engine NoOps inserted just before."""
    import json

    j = json.loads(bj)
    ctr = 0
    for f in j["functions"]:
        for bb in f["blocks"]:
            out = []
            for inst in bb["instructions"]:
                si = inst.get("sync_info")
                cap = 1
                waits = (si or {}).get("on_wait") or []
                if len(waits) > cap:
                    extra, keep = waits[:-cap], waits[-cap:]
                    for g in range(0, len(extra), 1):
                        ctr += 1
                        out.append({
                            "debug": inst.get("debug", 0),
                            "engine": inst["engine"],
                            "ins": [],
                            "name": f"WS-{ctr}",
                            "opcode": "NoOp",
                            "outs": [],
                            "sync_info": {"on_update": [],
                                          "on_wait": extra[g:g + 1]},
                        })
                    si["on_wait"] = keep
                out.append(inst)
            bb["instructions"] = out
    return json.dumps(j).encode()


def _build_nc():
    from contextlib import ExitStack

    import concourse.bass as bass
    import concourse.tile as tile
    from concourse import mybir

    F32 = mybir.dt.float32
    F16 = mybir.dt.float16 if DT16 == "fp16" else mybir.dt.bfloat16
    U8 = mybir.dt.uint8

    nc = bass.Bass(trn_type="TRN2", target_bir_lowering=False)
    srcA = nc.dram_tensor("srcA", [96, RAST], F16, kind="ExternalInput")
    idxu = nc.dram_tensor("idxu", [128, TOT], U8, kind="ExternalInput")
    wtA = nc.dram_tensor("wtA", [96, 128], F16, kind="ExternalInput")
    wtB = nc.dram_tensor("wtB", [48, 128], F16, kind="ExternalInput")
    iotain = nc.dram_tensor("iotain", [128, 1], F32, kind="ExternalInput")
    o = nc.dram_tensor("o", [32, CH], F32, kind="ExternalOutput")

    with tile.TileContext(nc) as tc, ExitStack() as ctx:
        sb = ctx.enter_context(tc.tile_pool(name="sb", bufs=1))
        cv_pool = ctx.enter_context(tc.tile_pool(name="cv", bufs=2))
        m_pool = ctx.enter_context(tc.tile_pool(name="m", bufs=3))
        psc_pool = ctx.enter_context(tc.tile_pool(name="psc", bufs=2, space="PSUM"))
        pso_pool = ctx.enter_context(tc.tile_pool(name="pso", bufs=2, space="PSUM"))

        iota_f = sb.tile([128, 1], F32)
        # warmup rhs first: the PE dummies block on it
        dumrhs = sb.tile([128, CH], F16)
        nc.vector.memset(dumrhs[:], 0.0)
        # sliding-window reduce lhsT: ones only in col 31; slice [31-c, 63-c)
        onewin = sb.tile([128, 63], F16)
        nc.vector.memset(onewin[:], 0.0)
        nc.vector.memset(onewin[:, 31:32], 1.0)

        wtA_t = sb.tile([96, 128], F16)
        wtB_t = sb.tile([48, 128], F16)

        btileA = sb.tile([96, BPAD], F16)
        nc.vector.memset(btileA[:, RAST:], 0.0)
        idx_t = sb.tile([128, TOT], U8)
        osb_a = sb.tile([16, CH], F32)
        osb_b = sb.tile([16, CH], F32)

        # DMAs: btileA is the startup critical path — it streams on the
        # gpsimd SWDGE queue, which measures ~4x the per-queue bandwidth
        # of the SP/ACT HWDGE queues. idx (first needed ~10us in) streams
        # on SP/ACT alternating. Weights go first on SP (tiny).
        nc.sync.dma_start(wtA_t[:], wtA.ap())
        nc.sync.dma_start(btileA[:, 0:512], srcA.ap()[:, 0:512])
        nc.sync.dma_start(wtB_t[:], wtB.ap())
        nc.scalar.dma_start(iota_f[:], iotain.ap())
        nc.scalar.dma_start(btileA[:, 512:1024], srcA.ap()[:, 512:1024])
        # SWDGE issue costs ~1us each: fewer, wider pieces so delivery of
        # the chunk 2-7 columns outruns the PE instead of trailing it.
        SG = [1024, 2560, 4608, 6656, 8704, 10752, 12800, 14848, RAST]
        for k in range(len(SG) - 1):
            nc.gpsimd.dma_start(btileA[:, SG[k]:SG[k + 1]],
                                srcA.ap()[:, SG[k]:SG[k + 1]])
        IX = list(range(0, TOT + 1, 2048))
        for k in range(len(IX) - 1):
            eng = nc.sync if k % 2 == 0 else nc.scalar
            eng.dma_start(idx_t[:, IX[k]:IX[k + 1]],
                          idxu.ap()[:, IX[k]:IX[k + 1]])

        # PE warmup: keep the PE busy while the first DMAs land. Targets
        # pso_b row 0, which the first real reduce into pso_b resets.
        pso_a = pso_pool.tile([16, CH], F32)
        pso_b = pso_pool.tile([16, CH], F32)
        for _ in range(8):
            nc.tensor.matmul(pso_b[0:1, :], lhsT=onewin[:, 31:32],
                             rhs=dumrhs[:], start=True, stop=True)

        # Reduce matmuls are issued 2 groups behind the conv matmuls: the
        # PE stream is in-order, so R(g) right after B(g) would stall the
        # PE on the ACT->DVE select chain. Two groups of A/B work hide
        # that chain latency. Chunks 0-15 accumulate rows into pso_a,
        # 16-31 into pso_b, so the first half's evacuation + output DMA
        # overlap the second half's compute.
        LAG = 2
        mtiles = {}

        def emit_reduce(g):
            m = mtiles.pop(g)
            for s in range(2):
                c = 2 * g + s
                pso = pso_a if c < 16 else pso_b
                r = c % 16
                nc.tensor.matmul(pso[:], lhsT=onewin[:, 31 - r:47 - r],
                                 rhs=m[:, s * CH:(s + 1) * CH],
                                 start=(r == 0), stop=(r == 15))

        for g in range(NCH // 2):
            c0, c1 = 2 * g * CH, (2 * g + 1) * CH
            psc = psc_pool.tile([128, 2 * CH], F32)
            # snake order A0 B0 | B1 [R R] A1: consecutive same-lhsT
            # matmuls (B0-B1, A1-next A0) skip the serial LDWEIGHTS.
            nc.tensor.matmul(psc[:, 0:CH], lhsT=wtA_t[:],
                             rhs=btileA[0:96, c0:c0 + CH],
                             start=True, stop=False)
            nc.tensor.matmul(psc[:, 0:CH], lhsT=wtB_t[:],
                             rhs=btileA[0:48, c0 + 2:c0 + 2 + CH],
                             start=False, stop=True)
            nc.tensor.matmul(psc[:, CH:2 * CH], lhsT=wtB_t[:],
                             rhs=btileA[0:48, c1 + 2:c1 + 2 + CH],
                             start=True, stop=False)
            if g >= LAG:
                emit_reduce(g - LAG)
            nc.tensor.matmul(psc[:, CH:2 * CH], lhsT=wtA_t[:],
                             rhs=btileA[0:96, c1:c1 + CH],
                             start=False, stop=True)
            if g == 9:
                # chunks 0..15 are reduced by now (groups 0..7 emitted);
                # drain the first half while the second half computes.
                nc.scalar.copy(osb_a[:], pso_a[:])
                nc.sync.dma_start(o.ap()[0:16, :], osb_a[:])
            cv = cv_pool.tile([128, 2 * CH], F16)
            m = m_pool.tile([128, 2 * CH], F16)
            mtiles[g] = m
            # last group: chunk-granular ACT/DVE so the drain chain after
            # the final conv matmul is half as long
            spans = ([(0, CH), (CH, 2 * CH)] if g == NCH // 2 - 1
                     else [(0, 2 * CH)])
            for lo, hi in spans:
                nc.scalar.copy(cv[:, lo:hi], psc[:, lo:hi])
                nc.vector.scalar_tensor_tensor(
                    out=m[:, lo:hi],
                    in0=idx_t[:, 2 * g * CH + lo:2 * g * CH + hi],
                    scalar=iota_f[:], in1=cv[:, lo:hi],
                    op0=mybir.AluOpType.is_equal, op1=mybir.AluOpType.mult,
                )
        for g in range(NCH // 2 - LAG, NCH // 2):
            emit_reduce(g)
        nc.scalar.copy(osb_b[:], pso_b[:])
        nc.sync.dma_start(o.ap()[16:32, :], osb_b[:])

    orig = nc.to_json_bytes
    nc.to_json_bytes = lambda: _split_waits_json(orig())
    return nc


def _get_nc():
    if "nc" not in _NC_CACHE:
        _NC_CACHE["nc"] = _build_nc()
    return _NC_CACHE["nc"]


def _np16():
    if DT16 == "fp16":
        return np.float16
    from ml_dtypes import bfloat16
    return bfloat16


def _in_maps(data, kernel_idx, weights):
    B = data.shape[0]
    np16 = _np16()
    # wtA[g*16+c, j] = W[j, c, dy_g, dx_g]; wtB[dy*16+c, j] = W[j, c, dy, 2]
    wA = np.empty((96, 128), np16)
    for gi, (dy, dx) in enumerate(_OFFS):
        wA[gi * 16:(gi + 1) * 16] = weights[:, :, dy, dx].T.astype(np16)
    wB = np.empty((48, 128), np16)
    for dy in range(3):
        wB[dy * 16:(dy + 1) * 16] = weights[:, :, dy, 2].T.astype(np16)
    iota = np.arange(128, dtype=np.float32).reshape(128, 1)
    maps = []
    for b in range(B):
        dpad = np.zeros((16, 128, 129), np.float32)
        dpad[:, :, :128] = data[b]
        sA = np.empty((96, RAST), np16)
        for gi, (dy, dx) in enumerate(_OFFS):
            sA[gi * 16:(gi + 1) * 16] = (
                dpad[:, dy:dy + 126, dx:dx + 128].reshape(16, RAST)
                .astype(np16))
        idxr = np.full((126, 128), 255, np.uint8)
        idxr[:, :126] = kernel_idx[b]
        flat = np.full(TOT, 255, np.uint8)
        flat[:RAST] = idxr.reshape(RAST)
        idxb = np.ascontiguousarray(np.broadcast_to(flat, (128, TOT)))
        maps.append({
            "srcA": sA,
            "idxu": idxb,
            "wtA": wA,
            "wtB": wB,
            "iotain": iota,
        })
    return maps


def kernel(data, kernel_idx, weights, _trace=False):
    from concourse.bass_utils import run_bass_kernel_spmd

    data = np.asarray(data, dtype=np.float32)
    kernel_idx = np.asarray(kernel_idx)
    weights = np.asarray(weights, dtype=np.float32)
    B = data.shape[0]
    nc = _get_nc()
    res = run_bass_kernel_spmd(nc, _in_maps(data, kernel_idx, weights),
                               core_ids=list(range(B)), trace=_trace)
    out = np.stack([
        r["o"].reshape(TOT)[:RAST].reshape(126, 128)[:, :126]
        for r in res.results
    ])
    if _trace:
        return out.astype(np.float32), res
    return out.astype(np.float32)


# revision 6
# speedup vs baseline: 1.0234x; 1.0234x over previous
"""Per-pixel kernel-lookup conv for trn2, data-parallel over batch on 8 cores.

Per core (one image): conv against all 128 kernels via 2 matmuls per
512-col chunk (A: K=96 = 16ch x {(dy,0),(dy,1)} materialized shifts;
B: K=48 dx=2 via +2 column offset on the dx=0 rows), ACT evacuates
PSUM->fp16, DVE does (idx==j)*conv select, a sliding-window ones-column
matmul partition-reduces each chunk into its own PSUM row, single ACT
evac + DMA out at the end.
"""
import os

import numpy as np

DT16 = os.environ.get("V2DT", "fp16")  # fp16 | bf16 for matmul operands
RAST = 126 * 128          # valid output raster (126 rows x 128 padded cols)
CH = 512                  # chunk cols (one PSUM bank of f32)
NCH = 32                  # chunks (covers 16384 >= RAST, tail is don't-care)
TOT = CH * NCH            # 16384
BPAD = 16512              # btileA cols: TOT + 2 (dx shift) rounded up
_NC_CACHE = {}

_OFFS = [(0, 0), (1, 0), (2, 0), (0, 1), (1, 1), (2, 1)]  # btileA row groups


def _split_waits_json(bj: bytes) -> bytes:
    """Walrus rejects >4 sync-waits per instruction (and ~2 on Matmult).
    Split excess waits onto same-engine NoOps inserted just before."""
    import json

    j = json.loads(bj)
    ctr = 0
    for f in j["functions"]:
        for bb in f["blocks"]:
            out = []
            for inst in bb["instructions"]:
                si = inst.get("sync_info")
                cap = 1
                waits = (si or {}).get("on_wait") or []
                if len(waits) > cap:
                    extra, keep = waits[:-cap], waits[-cap:]
                    for g in range(0, len(extra), 1):
                        ctr += 1
                        out.append({
                            "debug": inst.get("debug", 0),
                            "engine": inst["engine"],
                            "ins": [],
                            "name": f"WS-{ctr}",
                            "opcode": "NoOp",
                            "outs": [],
                            "sync_info": {"on_update": [],
                                          "on_wait": extra[g:g + 1]},
                        })
                    si["on_wait"] = keep
                out.append(inst)
            bb["instructions"] = out
    return json.dumps(j).encode()


def _build_nc():
    from contextlib import ExitStack

    import concourse.bass as bass
    import concourse.tile as tile
    from concourse import mybir

    F32 = mybir.dt.float32
    F16 = mybir.dt.float16 if DT16 == "fp16" else mybir.dt.bfloat16
    U8 = mybir.dt.uint8

    nc = bass.Bass(trn_type="TRN2", target_bir_lowering=False)
    srcA = nc.dram_tensor("srcA", [96, RAST], F16, kind="ExternalInput")
    idxu = nc.dram_tensor("idxu", [128, TOT], U8, kind="ExternalInput")
    wtA = nc.dram_tensor("wtA", [96, 128], F16, kind="ExternalInput")
    wtB = nc.dram_tensor("wtB", [48, 128], F16, kind="ExternalInput")
    iotain = nc.dram_tensor("iotain", [128, 1], F32, kind="ExternalInput")
    o = nc.dram_tensor("o", [32, CH], F32, kind="ExternalOutput")

    with tile.TileContext(nc) as tc, ExitStack() as ctx:
        sb = ctx.enter_context(tc.tile_pool(name="sb", bufs=1))
        cv_pool = ctx.enter_context(tc.tile_pool(name="cv", bufs=2))
        m_pool = ctx.enter_context(tc.tile_pool(name="m", bufs=3))
        psc_pool = ctx.enter_context(tc.tile_pool(name="psc", bufs=2, space="PSUM"))
        pso_pool = ctx.enter_context(tc.tile_pool(name="pso", bufs=2, space="PSUM"))

        iota_f = sb.tile([128, 1], F32)
        # warmup rhs first: the PE dummies block on it
        dumrhs = sb.tile([128, CH], F16)
        nc.vector.memset(dumrhs[:], 0.0)
        # sliding-window reduce lhsT: ones only in col 31; slice [31-c, 63-c)
        onewin = sb.tile([128, 63], F16)
        nc.vector.memset(onewin[:], 0.0)
        nc.vector.memset(onewin[:, 31:32], 1.0)

        wtA_t = sb.tile([96, 128], F16)
        wtB_t = sb.tile([48, 128], F16)

        btileA = sb.tile([96, BPAD], F16)
        nc.vector.memset(btileA[:, RAST:], 0.0)
        idx_t = sb.tile([128, TOT], U8)
        osb_a = sb.tile([16, CH], F32)
        osb_b = sb.tile([16, CH], F32)

        # DMAs: btileA is the startup critical path — it streams on the
        # gpsimd SWDGE queue, which measures ~4x the per-queue bandwidth
        # of the SP/ACT HWDGE queues. idx (first needed ~10us in) streams
        # on SP/ACT alternating. Weights go first on SP (tiny).
        SA = ([0, 512, 1024, 1536, 2048, 2560, 3072, 3584] +
              list(range(5632, RAST, 2048)) + [RAST])
        nc.sync.dma_start(wtA_t[:], wtA.ap())
        nc.sync.dma_start(btileA[:, SA[0]:SA[1]], srcA.ap()[:, SA[0]:SA[1]])
        nc.sync.dma_start(wtB_t[:], wtB.ap())
        nc.scalar.dma_start(iota_f[:], iotain.ap())
        nc.scalar.dma_start(btileA[:, SA[1]:SA[2]], srcA.ap()[:, SA[1]:SA[2]])
        for k in range(2, len(SA) - 1):
            nc.gpsimd.dma_start(btileA[:, SA[k]:SA[k + 1]],
                                srcA.ap()[:, SA[k]:SA[k + 1]])
        IX = list(range(0, TOT + 1, 2048))
        for k in range(len(IX) - 1):
            eng = nc.sync if k % 2 == 0 else nc.scalar
            eng.dma_start(idx_t[:, IX[k]:IX[k + 1]],
                          idxu.ap()[:, IX[k]:IX[k + 1]])

        # PE warmup: keep the PE busy while the first DMAs land. Targets
        # pso_b row 0, which the first real reduce into pso_b resets.
        pso_a = pso_pool.tile([16, CH], F32)
        pso_b = pso_pool.tile([16, CH], F32)
        for _ in range(8):
            nc.tensor.matmul(pso_b[0:1, :], lhsT=onewin[:, 31:32],
                             rhs=dumrhs[:], start=True, stop=True)

        # Reduce matmuls are issued 2 groups behind the conv matmuls: the
        # PE stream is in-order, so R(g) right after B(g) would stall the
        # PE on the ACT->DVE select chain. Two groups of A/B work hide
        # that chain latency. Chunks 0-15 accumulate rows into pso_a,
        # 16-31 into pso_b, so the first half's evacuation + output DMA
        # overlap the second half's compute.
        LAG = 2
        mtiles = {}

        def emit_reduce(g):
            m = mtiles.pop(g)
            for s in range(2):
                c = 2 * g + s
                pso = pso_a if c < 16 else pso_b
                r = c % 16
                nc.tensor.matmul(pso[:], lhsT=onewin[:, 31 - r:47 - r],
                                 rhs=m[:, s * CH:(s + 1) * CH],
                                 start=(r == 0), stop=(r == 15))

        for g in range(NCH // 2):
            c0, c1 = 2 * g * CH, (2 * g + 1) * CH
            psc = psc_pool.tile([128, 2 * CH], F32)
            # snake order A0 B0 | B1 [R R] A1: consecutive same-lhsT
            # matmuls (B0-B1, A1-next A0) skip the serial LDWEIGHTS.
            nc.tensor.matmul(psc[:, 0:CH], lhsT=wtA_t[:],
                             rhs=btileA[0:96, c0:c0 + CH],
                             start=True, stop=False)
            nc.tensor.matmul(psc[:, 0:CH], lhsT=wtB_t[:],
                             rhs=btileA[0:48, c0 + 2:c0 + 2 + CH],
                             start=False, stop=True)
            nc.tensor.matmul(psc[:, CH:2 * CH], lhsT=wtB_t[:],
                             rhs=btileA[0:48, c1 + 2:c1 + 2 + CH],
                             start=True, stop=False)
            if g >= LAG:
                emit_reduce(g - LAG)
            nc.tensor.matmul(psc[:, CH:2 * CH], lhsT=wtA_t[:],
                             rhs=btileA[0:96, c1:c1 + CH],
                             start=False, stop=True)
            if g == 9:
                # chunks 0..15 are reduced by now (groups 0..7 emitted);
                # drain the first half while the second half computes.
                nc.scalar.copy(osb_a[:], pso_a[:])
                nc.sync.dma_start(o.ap()[0:16, :], osb_a[:])
            cv = cv_pool.tile([128, 2 * CH], F16)
            m = m_pool.tile([128, 2 * CH], F16)
            mtiles[g] = m
            # last group: chunk-granular ACT/DVE so the drain chain after
            # the final conv matmul is half as long
            spans = ([(0, CH), (CH, 2 * CH)] if g == NCH // 2 - 1
                     else [(0, 2 * CH)])
            for lo, hi in spans:
                nc.scalar.copy(cv[:, lo:hi], psc[:, lo:hi])
                nc.vector.scalar_tensor_tensor(
                    out=m[:, lo:hi],
                    in0=idx_t[:, 2 * g * CH + lo:2 * g * CH + hi],
                    scalar=iota_f[:], in1=cv[:, lo:hi],
                    op0=mybir.AluOpType.is_equal, op1=mybir.AluOpType.mult,
                )
        for g in range(NCH // 2 - LAG, NCH // 2):
            emit_reduce(g)
        nc.scalar.copy(osb_b[:], pso_b[:])
        nc.sync.dma_start(o.ap()[16:32, :], osb_b[:])

    orig = nc.to_json_bytes
    nc.to_json_bytes = lambda: _split_waits_json(orig())
    return nc


def _get_nc():
    if "nc" not in _NC_CACHE:
        _NC_CACHE["nc"] = _build_nc()
    return _NC_CACHE["nc"]


def _np16():
    if DT16 == "fp16":
        return np.float16
    from ml_dtypes import bfloat16
    return bfloat16


def _in_maps(data, kernel_idx, weights):
    B = data.shape[0]
    np16 = _np16()
    # wtA[g*16+c, j] = W[j, c, dy_g, dx_g]; wtB[dy*16+c, j] = W[j, c, dy, 2]
    wA = np.empty((96, 128), np16)
    for gi, (dy, dx) in enumerate(_OFFS):
        wA[gi * 16:(gi + 1) * 16] = weights[:, :, dy, dx].T.astype(np16)
    wB = np.empty((48, 128), np16)
    for dy in range(3):
        wB[dy * 16:(dy + 1) * 16] = weights[:, :, dy, 2].T.astype(np16)
    iota = np.arange(128, dtype=np.float32).reshape(128, 1)
    maps = []
    for b in range(B):
        dpad = np.zeros((16, 128, 129), np.float32)
        dpad[:, :, :128] = data[b]
        sA = np.empty((96, RAST), np16)
        for gi, (dy, dx) in enumerate(_OFFS):
            sA[gi * 16:(gi + 1) * 16] = (
                dpad[:, dy:dy + 126, dx:dx + 128].reshape(16, RAST)
                .astype(np16))
        idxr = np.full((126, 128), 255, np.uint8)
        idxr[:, :126] = kernel_idx[b]
        flat = np.full(TOT, 255, np.uint8)
        flat[:RAST] = idxr.reshape(RAST)
        idxb = np.ascontiguousarray(np.broadcast_to(flat, (128, TOT)))
        maps.append({
            "srcA": sA,
            "idxu": idxb,
            "wtA": wA,
            "wtB": wB,
            "iotain": iota,
        })
    return maps


def kernel(data, kernel_idx, weights, _trace=False):
    from concourse.bass_utils import run_bass_kernel_spmd

    data = np.asarray(data, dtype=np.float32)
    kernel_idx = np.asarray(kernel_idx)
    weights = np.asarray(weights, dtype=np.float32)
    B = data.shape[0]
    nc = _get_nc()
    res = run_bass_kernel_spmd(nc, _in_maps(data, kernel_idx, weights),
                               core_ids=list(range(B)), trace=_trace)
    out = np.stack([
        r["o"].reshape(TOT)[:RAST].reshape(126, 128)[:, :126]
        for r in res.results
    ])
    if _trace:
        return out.astype(np.float32), res
    return out.astype(np.float32)
